# revision 10
# baseline (speedup 1.0000x reference)
"""Causal self-attention (B=4, S=2048, E=1024, H=16) on 8 trn2 cores.

Sharding: tensor-parallel over heads. Core c owns heads (2c, 2c+1):
  - computes q,k,v for its 2 heads from the full x (qkv matmul with its
    128-column slice of W_attn),
  - runs causal attention for those heads,
  - multiplies by its 128-row slice of W_proj producing a PARTIAL [T, E]
    output; the host sums the 8 partials and adds b_proj.

Device layout notes:
  - x is passed pre-transposed (xT [E, T]) so the contraction dim E lands on
    SBUF partitions for the qkv matmuls.
  - q,k are kept transposed (QT/KT [head-col, token]) which is exactly the
    lhsT/rhs layout needed for scores^T = K @ Q^T (contraction over D=64).
  - softmax runs on scores^T [k, q] without max-subtraction (scores are O(7)
    for this data, exp is safe in f32); the row-sum falls out of the w@V
    matmul via an extra all-ones column appended to V.
  - y^T = V_aug^T @ exp(s^T) gives [d, q] plus the sum row, normalized with a
    reciprocal broadcast, and is directly the lhsT for the projection.
"""

import sys

if "/opt/trn_rl_repo" not in sys.path:
    sys.path.insert(0, "/opt/trn_rl_repo")

import numpy as np

N_EMBD = 1024
N_HEAD = 16
D = 64
N_CORES = 8
HPC = N_HEAD // N_CORES  # heads per core = 2
B_FULL = 4
S_FULL = 2048

# run matmul operands as float32r (reduced-precision multiply, ~4x faster).
# fp32r data must be *stored* rounded, so this switches the dtype of every
# tensor feeding a matmul; biases/masks/softmax sums stay exact f32.
USE_F32R = True


def _patch_tile(tile):
    """This container's walrus build allows max 1 sem wait per instruction;
    stock Tile can attach several (tail drain, and any instruction whose
    inputs come from 2+ engines/queues). Split extras onto standalone
    single-wait nop carriers on the same engine, emitted just before."""
    if getattr(tile.TileContext, "_drain_split_patched", False):
        return

    orig_commit = tile.TileContext._commit_instruction

    def _commit_instruction(self, inst, lazy_reg_writes=True):
        si = inst.sync_info
        waits = list(si.on_wait) if si is not None and si.on_wait else []
        if len(waits) > 1:
            by_name = {h.name: h for h in self.sems.allocated().values()}
            for w in waits[:-1]:
                h = by_name.get(w.ant_name)
                if h is None:
                    raise RuntimeError(f"wait-split: no handle for {w.ant_name}")
                nop = self.nc.engines[inst.engine].nop(nofuse=True)
                nop.wait_op(h, w.wait_value, _wait_mode_op(w), check=False)
            inst.sync_info.on_wait = [waits[-1]]
        return orig_commit(self, inst, lazy_reg_writes)

    def _wait_mode_op(w):
        m = str(w.wait_mode)
        if "ge" in m:
            return "sem-ge"
        if "eq" in m:
            return "sem-eq"
        raise RuntimeError(f"wait-split: unsupported wait mode {m}")

    tile.TileContext._commit_instruction = _commit_instruction

    def _drain_and_barrier(self, tick_clock, wait_clock):
        nc = self.nc
        drain_inst = nc.sync.drain()
        wait_clock.add_sem_waits(
            drain_inst.ins, tile.ScopedClock({None: tick_clock.global_clock})
        )
        waits = list(drain_inst.ins.sync_info.on_wait or [])
        if len(waits) > 1:
            drain_inst.ins.sync_info.on_wait = [waits[0]]
            by_name = {}
            if self.sems is not None:
                by_name = {h.name: h for h in self.sems.allocated().values()}
            for w in waits[1:]:
                extra = nc.sync.drain()
                h = by_name.get(w.ant_name)
                if h is None:
                    raise RuntimeError(f"drain-split: no handle for {w.ant_name}")
                extra._wait_ge(h, w.wait_value)
        nc.all_engine_barrier()
        assert self.sems is not None
        popped = nc._tile_sem_poison_stack.pop()
        assert popped is self._sem_poison
        nc.clear_and_free_semaphores(list(self.sems.allocated().values()))
        nc.all_engine_barrier()

    tile.TileContext._drain_and_barrier = _drain_and_barrier
    tile.TileContext._drain_split_patched = True


def build_nc(nb=B_FULL, s=S_FULL, num_devices=N_CORES):
    import concourse.bass as bass
    import concourse.mybir as mybir
    import concourse.tile as tile
    from concourse.bass import ds, ts
    from concourse.masks import make_identity

    _patch_tile(tile)

    f32 = mybir.dt.float32
    f32r = mybir.dt.float32r
    AF = mybir.ActivationFunctionType
    E = N_EMBD
    T = nb * s
    KO = E // 128  # contraction chunks for qkv
    NT = s // 512  # token 512-tiles per batch
    NKC = s // 128  # k 128-chunks per batch
    assert s % 512 == 0

    rdt = f32r if USE_F32R else f32

    nc = bass.Bass(
        "TRN2", target_bir_lowering=False, debug=False, num_devices=num_devices
    )
    xT = nc.dram_tensor("xT", [E, T], rdt, kind="ExternalInput")
    Wq = nc.dram_tensor("Wq", [E, 128], rdt, kind="ExternalInput")
    Wk = nc.dram_tensor("Wk", [E, 128], rdt, kind="ExternalInput")
    Wv = nc.dram_tensor("Wv", [E, 128], rdt, kind="ExternalInput")
    bq = nc.dram_tensor("bq", [128], f32, kind="ExternalInput")
    bk = nc.dram_tensor("bk", [128], f32, kind="ExternalInput")
    bv = nc.dram_tensor("bv", [128], f32, kind="ExternalInput")
    Wp = nc.dram_tensor("Wp", [128, E], rdt, kind="ExternalInput")
    masks = nc.dram_tensor("masks", [4, 128, 512], f32, kind="ExternalInput")
    out = nc.dram_tensor("out", [T, E], f32, kind="ExternalOutput")

    xT_r = xT[:].rearrange("(ko p) t -> p ko t", p=128)

    from contextlib import ExitStack

    with tile.TileContext(nc) as tc, ExitStack() as ctx:
        const = ctx.enter_context(tc.tile_pool(name="const", bufs=1))
        pb = ctx.enter_context(tc.tile_pool(name="perb", bufs=2))
        xp = ctx.enter_context(tc.tile_pool(name="xp", bufs=2))
        work = ctx.enter_context(tc.tile_pool(name="work", bufs=3))
        mm_ps = ctx.enter_context(tc.tile_pool(name="mmps", bufs=3, space="PSUM"))
        yt_ps = ctx.enter_context(tc.tile_pool(name="ytps", bufs=2, space="PSUM"))
        tp_ps = ctx.enter_context(tc.tile_pool(name="tpps", bufs=2, space="PSUM"))

        # constants
        Wq_sb = const.tile([128, KO, 128], rdt, tag="wq")
        Wk_sb = const.tile([128, KO, 128], rdt, tag="wk")
        Wv_sb = const.tile([128, KO, 128], rdt, tag="wv")
        nc.sync.dma_start(Wq_sb[:], Wq[:].rearrange("(ko p) c -> p ko c", p=128))
        nc.sync.dma_start(Wk_sb[:], Wk[:].rearrange("(ko p) c -> p ko c", p=128))
        nc.sync.dma_start(Wv_sb[:], Wv[:].rearrange("(ko p) c -> p ko c", p=128))
        Wp_sb = const.tile([128, E], rdt, tag="wp")
        nc.sync.dma_start(Wp_sb[:], Wp[:])
        bq_sb = const.tile([128, 1], f32, tag="bq")
        bk_sb = const.tile([128, 1], f32, tag="bk")
        bv_sb = const.tile([128, 1], f32, tag="bv")
        nc.sync.dma_start(bq_sb[:], bq[:].unsqueeze(1))
        nc.sync.dma_start(bk_sb[:], bk[:].unsqueeze(1))
        nc.sync.dma_start(bv_sb[:], bv[:].unsqueeze(1))
        mask_sb = const.tile([128, 4, 512], f32, tag="mask")
        nc.sync.dma_start(mask_sb[:], masks[:].rearrange("m p j -> p m j"))
        ident_f32 = const.tile([128, 128], f32, tag="ident_f32")
        make_identity(nc, ident_f32[:])
        ident = const.tile([128, 128], rdt, tag="ident")
        nc.vector.tensor_copy(ident[:], ident_f32[:])
        ones64 = const.tile([128, D], f32, tag="ones64")
        nc.gpsimd.memset(ones64[:], 1.0)

        for b in range(nb):
            base = b * s
            QT = pb.tile([128, s], rdt, tag="qt")
            KT = pb.tile([128, s], rdt, tag="kt")
            VT = pb.tile([128, s], rdt, tag="vt")
            Vaug = pb.tile([128, NKC, 2 * (D + 1)], rdt, tag="vaug")
            yTn = pb.tile([128, s], rdt, tag="ytn")
            # ones columns for the softmax row-sum (f32r via DVE convert)
            nc.vector.tensor_copy(Vaug[:, :, D], ones64[:, 0:NKC])
            nc.vector.tensor_copy(Vaug[:, :, 2 * D + 1], ones64[:, 0:NKC])

            # ---- qkv: QT/KT/VT[c, tok] = (x @ W*)^T + b ----
            for nt in range(NT):
                xt = xp.tile([128, KO, 512], rdt, tag="xt")
                nc.sync.dma_start(xt[:], xT_r[:, :, ds(base + nt * 512, 512)])
                for Wsb, dst, bias_sb, scale in (
                    (Wq_sb, QT, bq_sb, 0.125),
                    (Wk_sb, KT, bk_sb, 1.0),
                    (Wv_sb, VT, bv_sb, 1.0),
                ):
                    ps = mm_ps.tile([128, 512], f32, tag="mm")
                    for ko in range(KO):
                        nc.tensor.matmul(
                            ps[:],
                            lhsT=Wsb[:, ko],
                            rhs=xt[:, ko],
                            start=(ko == 0),
                            stop=(ko == KO - 1),
                        )
                    nc.scalar.activation(
                        dst[:, ts(nt, 512)],
                        ps[:],
                        AF.Identity,
                        bias=bias_sb[:],
                        scale=scale,
                    )

            # ---- V back to natural layout, with ones column per head ----
            for tt in range(NKC):
                for h in range(HPC):
                    tp = tp_ps.tile([128, D], rdt, tag="tp")
                    nc.tensor.transpose(
                        tp[:],
                        VT[ds(h * D, D), ts(tt, 128)],
                        ident[ds(h * D, D), ds(h * D, D)],
                    )
                    nc.vector.tensor_copy(Vaug[:, tt, ds(h * (D + 1), D)], tp[:])

            # ---- causal attention per head ----
            for h in range(HPC):
                hp = h * D
                for qt in range(NT):
                    nchunks = 4 * qt + 4
                    ytp = yt_ps.tile([D + 1, 512], f32, tag="yt")
                    for kc in range(nchunks):
                        sp = mm_ps.tile([128, 512], f32, tag="mm")
                        nc.tensor.matmul(
                            sp[:],
                            lhsT=KT[ds(hp, D), ts(kc, 128)],
                            rhs=QT[ds(hp, D), ts(qt, 512)],
                            start=True,
                            stop=True,
                        )
                        et = work.tile([128, 512], rdt, tag="exp")
                        nc.scalar.activation(et[:], sp[:], AF.Exp)
                        m = kc - 4 * qt
                        if m >= 0:
                            nc.vector.tensor_mul(et[:], et[:], mask_sb[:, m, :])
                        nc.tensor.matmul(
                            ytp[:],
                            lhsT=Vaug[:, kc, ds(h * (D + 1), D + 1)],
                            rhs=et[:],
                            start=(kc == 0),
                            stop=(kc == nchunks - 1),
                        )
                    # normalize: recip of sum row, replicate across 64
                    # partitions with an exact-f32 ones-matmul, multiply
                    rec = work.tile([D + 1, 512], f32, tag="rec")
                    nc.vector.reciprocal(rec[ds(D, 1), :], ytp[ds(D, 1), :])
                    rep_ps = tp_ps.tile([D, 512], f32, tag="tp")
                    nc.tensor.matmul(
                        rep_ps[:],
                        lhsT=ones64[ds(D, 1), :],
                        rhs=rec[ds(D, 1), :],
                        start=True,
                        stop=True,
                    )
                    rep = work.tile([D, 512], f32, tag="rep")
                    nc.scalar.activation(rep[:], rep_ps[:], AF.Copy)
                    nc.vector.tensor_mul(
                        yTn[ds(hp, D), ts(qt, 512)], ytp[0:D, :], rep[:]
                    )

            # ---- projection partial: out[tok, :] = y_norm @ Wp_slice ----
            for tt in range(NKC):
                ob = work.tile([128, E], f32, tag="ob")
                for n in range(E // 512):
                    op = mm_ps.tile([128, 512], f32, tag="mm")
                    nc.tensor.matmul(
                        op[:],
                        lhsT=yTn[:, ts(tt, 128)],
                        rhs=Wp_sb[:, ts(n, 512)],
                        start=True,
                        stop=True,
                    )
                    nc.scalar.activation(ob[:, ts(n, 512)], op[:], AF.Copy)
                nc.sync.dma_start(out[ds(base + tt * 128, 128), :], ob[:])

    return nc


def make_masks():
    i = np.arange(128)[:, None]
    j = np.arange(512)[None, :]
    return np.stack(
        [(j >= i + 128 * m).astype(np.float32) for m in range(4)], axis=0
    )


def shard_inputs(x, W_attn, b_attn, W_proj, nb, s):
    """Build the per-core input maps."""
    E = N_EMBD
    T = nb * s
    x2d = np.ascontiguousarray(x.reshape(T, E), dtype=np.float32)
    xT = np.ascontiguousarray(x2d.T)
    masks = make_masks()
    in_maps = []
    for c in range(N_CORES):
        lo = c * HPC * D
        hi = lo + HPC * D
        in_maps.append(
            {
                "xT": xT,
                "Wq": np.ascontiguousarray(W_attn[:, lo:hi]),
                "Wk": np.ascontiguousarray(W_attn[:, E + lo : E + hi]),
                "Wv": np.ascontiguousarray(W_attn[:, 2 * E + lo : 2 * E + hi]),
                "bq": np.ascontiguousarray(b_attn[lo:hi]) * 0.125,
                "bk": np.ascontiguousarray(b_attn[E + lo : E + hi]),
                "bv": np.ascontiguousarray(b_attn[2 * E + lo : 2 * E + hi]),
                "Wp": np.ascontiguousarray(W_proj[lo:hi, :]),
                "masks": masks,
            }
        )
    return in_maps


_NC_CACHE = {}


def _get_nc(nb, s):
    key = (nb, s)
    if key not in _NC_CACHE:
        _NC_CACHE[key] = build_nc(nb, s)
    return _NC_CACHE[key]


def kernel(x, W_attn, b_attn, W_proj, b_proj, _trace=False):
    from concourse.bass_utils import run_bass_kernel_spmd

    nb, s, E = x.shape
    assert E == N_EMBD
    nc = _get_nc(nb, s)
    in_maps = shard_inputs(x, W_attn, b_attn, W_proj, nb, s)
    res = run_bass_kernel_spmd(nc, in_maps, list(range(N_CORES)), trace=_trace)
    acc = res.results[0]["out"].astype(np.float32)
    for c in range(1, N_CORES):
        acc += res.results[c]["out"]
    acc += b_proj.astype(np.float32)
    out = acc.reshape(nb, s, E)
    kernel.last_results = res
    return out


# revision 29
# speedup vs baseline: 13.7165x; 13.7165x over previous
"""Causal self-attention (B=4, S=2048, E=1024, H=16) on 8 trn2 cores.

Sharding: tensor-parallel over heads. Core c owns heads (2c, 2c+1):
  - computes q,k,v for its 2 heads from the full x (qkv matmul with its
    128-column slice of W_attn),
  - runs causal attention for those heads,
  - multiplies by its 128-row slice of W_proj producing a PARTIAL [T, E]
    output; the host sums the 8 partials and adds b_proj.

Device layout notes:
  - x is passed pre-transposed (xT [E, T]) so the contraction dim E lands on
    SBUF partitions for the qkv matmuls.
  - q,k are kept transposed (QT/KT [head-col, token]) which is exactly the
    lhsT/rhs layout needed for scores^T = K @ Q^T (contraction over D=64).
  - softmax runs on scores^T [k, q] without max-subtraction (scores are O(7)
    for this data, exp is safe in f32); the row-sum falls out of the w@V
    matmul via an extra all-ones column appended to V.
  - y^T = V_aug^T @ exp(s^T) gives [d, q] plus the sum row, normalized with a
    reciprocal broadcast, and is directly the lhsT for the projection.
"""

import sys

if "/opt/trn_rl_repo" not in sys.path:
    sys.path.insert(0, "/opt/trn_rl_repo")

import numpy as np

N_EMBD = 1024
N_HEAD = 16
D = 64
N_CORES = 8
HPC = N_HEAD // N_CORES  # heads per core = 2
B_FULL = 4
S_FULL = 2048

# run matmul operands as float32r (reduced-precision multiply, ~4x faster).
# fp32r data must be *stored* rounded, so this switches the dtype of every
# tensor feeding a matmul; biases/masks/softmax sums stay exact f32.
USE_F32R = True


def _patch_tile(tile):
    """This container's walrus build allows max 1 sem wait per instruction;
    stock Tile can attach several (tail drain, and any instruction whose
    inputs come from 2+ engines/queues). Split extras onto standalone
    single-wait nop carriers on the same engine, emitted just before."""
    if getattr(tile.TileContext, "_drain_split_patched", False):
        return

    orig_commit = tile.TileContext._commit_instruction

    def _commit_instruction(self, inst, lazy_reg_writes=True):
        si = inst.sync_info
        waits = list(si.on_wait) if si is not None and si.on_wait else []
        if len(waits) > 1:
            by_name = {h.name: h for h in self.sems.allocated().values()}
            for w in waits[:-1]:
                h = by_name.get(w.ant_name)
                if h is None:
                    raise RuntimeError(f"wait-split: no handle for {w.ant_name}")
                nop = self.nc.engines[inst.engine].nop(nofuse=True)
                nop.wait_op(h, w.wait_value, _wait_mode_op(w), check=False)
            inst.sync_info.on_wait = [waits[-1]]
        return orig_commit(self, inst, lazy_reg_writes)

    def _wait_mode_op(w):
        m = str(w.wait_mode)
        if "ge" in m:
            return "sem-ge"
        if "eq" in m:
            return "sem-eq"
        raise RuntimeError(f"wait-split: unsupported wait mode {m}")

    tile.TileContext._commit_instruction = _commit_instruction

    def _drain_and_barrier(self, tick_clock, wait_clock):
        nc = self.nc
        drain_inst = nc.sync.drain()
        wait_clock.add_sem_waits(
            drain_inst.ins, tile.ScopedClock({None: tick_clock.global_clock})
        )
        waits = list(drain_inst.ins.sync_info.on_wait or [])
        if len(waits) > 1:
            drain_inst.ins.sync_info.on_wait = [waits[0]]
            by_name = {}
            if self.sems is not None:
                by_name = {h.name: h for h in self.sems.allocated().values()}
            for w in waits[1:]:
                extra = nc.sync.drain()
                h = by_name.get(w.ant_name)
                if h is None:
                    raise RuntimeError(f"drain-split: no handle for {w.ant_name}")
                extra._wait_ge(h, w.wait_value)
        nc.all_engine_barrier()
        assert self.sems is not None
        popped = nc._tile_sem_poison_stack.pop()
        assert popped is self._sem_poison
        nc.clear_and_free_semaphores(list(self.sems.allocated().values()))
        nc.all_engine_barrier()

    tile.TileContext._drain_and_barrier = _drain_and_barrier
    tile.TileContext._drain_split_patched = True


def build_nc(nb=B_FULL, s=S_FULL, num_devices=N_CORES):
    import concourse.bass as bass
    import concourse.mybir as mybir
    import concourse.tile as tile
    from concourse.bass import ds, ts
    from concourse.masks import make_identity

    _patch_tile(tile)

    f32 = mybir.dt.float32
    f32r = mybir.dt.float32r
    AF = mybir.ActivationFunctionType
    E = N_EMBD
    T = nb * s
    KO = E // 128  # contraction chunks for qkv
    NT = s // 512  # token 512-tiles per batch
    NKC = s // 128  # k 128-chunks per batch
    assert s % 512 == 0

    rdt = f32r if USE_F32R else f32

    nc = bass.Bass(
        "TRN2", target_bir_lowering=False, debug=False, num_devices=num_devices
    )
    xT = nc.dram_tensor("xT", [E, T], rdt, kind="ExternalInput")
    Wq = nc.dram_tensor("Wq", [E, 128], rdt, kind="ExternalInput")
    Wk = nc.dram_tensor("Wk", [E, 128], rdt, kind="ExternalInput")
    Wv = nc.dram_tensor("Wv", [E, 128], rdt, kind="ExternalInput")
    bq = nc.dram_tensor("bq", [128], f32, kind="ExternalInput")
    bk = nc.dram_tensor("bk", [128], f32, kind="ExternalInput")
    bv = nc.dram_tensor("bv", [128], f32, kind="ExternalInput")
    Wp = nc.dram_tensor("Wp", [128, E], rdt, kind="ExternalInput")
    out = nc.dram_tensor("out", [T, E], f32, kind="ExternalOutput")

    xT_r = xT[:].rearrange("(ko p) t -> p ko t", p=128)

    from contextlib import ExitStack

    with tile.TileContext(nc) as tc, ExitStack() as ctx:
        const = ctx.enter_context(tc.tile_pool(name="const", bufs=1))
        pb = ctx.enter_context(tc.tile_pool(name="perb", bufs=2))
        xp = ctx.enter_context(tc.tile_pool(name="xp", bufs=3))
        work = ctx.enter_context(tc.tile_pool(name="work", bufs=3))
        mm_ps = ctx.enter_context(tc.tile_pool(name="mmps", bufs=2, space="PSUM"))
        yt_ps = ctx.enter_context(tc.tile_pool(name="ytps", bufs=2, space="PSUM"))
        aux_ps = ctx.enter_context(tc.tile_pool(name="auxps", bufs=2, space="PSUM"))
        expp = ctx.enter_context(tc.tile_pool(name="expp", bufs=6))
        nrm = ctx.enter_context(tc.tile_pool(name="nrm", bufs=2))
        obp = ctx.enter_context(tc.tile_pool(name="obp", bufs=2))

        # constants
        Wq_sb = const.tile([128, KO, 128], rdt, tag="wq")
        Wk_sb = const.tile([128, KO, 128], rdt, tag="wk")
        Wv_sb = const.tile([128, KO, 128], rdt, tag="wv")
        nc.sync.dma_start(Wq_sb[:], Wq[:].rearrange("(ko p) c -> p ko c", p=128))
        nc.sync.dma_start(Wk_sb[:], Wk[:].rearrange("(ko p) c -> p ko c", p=128))
        nc.sync.dma_start(Wv_sb[:], Wv[:].rearrange("(ko p) c -> p ko c", p=128))
        Wp_sb = const.tile([128, E], rdt, tag="wp")
        nc.sync.dma_start(Wp_sb[:], Wp[:])
        bq_sb = const.tile([128, 1], f32, tag="bq")
        bk_sb = const.tile([128, 1], f32, tag="bk")
        bv_sb = const.tile([128, 1], f32, tag="bv")
        nc.sync.dma_start(bq_sb[:], bq[:].unsqueeze(1))
        nc.sync.dma_start(bk_sb[:], bk[:].unsqueeze(1))
        nc.sync.dma_start(bv_sb[:], bv[:].unsqueeze(1))
        ident_f32 = const.tile([128, 128], f32, tag="ident_f32")
        make_identity(nc, ident_f32[:])
        ident = const.tile([128, 128], rdt, tag="ident")
        nc.vector.tensor_copy(ident[:], ident_f32[:])
        ones64 = const.tile([128, D], f32, tag="ones64")
        nc.gpsimd.memset(ones64[:], 1.0)
        ones64r = const.tile([128, D], rdt, tag="ones64r")
        nc.vector.tensor_copy(ones64r[:], ones64[:])

        # Per-batch tiles live in `tiles[b]`; qkv for batch b+1 is emitted
        # interleaved with attention for batch b (one 512-token chunk per
        # attention q block) so the ACT-bound attention phase's spare PE
        # cycles absorb the next batch's PE-bound qkv matmuls.
        tiles = {}

        def alloc_batch(b):
            QT = pb.tile([128, s], rdt, tag="qt", name=f"QT{b}")
            KT = pb.tile([128, s], rdt, tag="kt", name=f"KT{b}")
            VT = pb.tile([128, s], rdt, tag="vt", name=f"VT{b}")
            Vaug = pb.tile(
                [128, NKC, 2 * (D + 1)], rdt, tag="vaug", name=f"Vaug{b}"
            )
            yTn = pb.tile([128, s], rdt, tag="ytn", name=f"yTn{b}")
            # ones columns for the softmax row-sum (f32r via DVE convert)
            nc.vector.tensor_copy(Vaug[:, :, D], ones64[:, 0:NKC])
            nc.vector.tensor_copy(Vaug[:, :, 2 * D + 1], ones64[:, 0:NKC])
            tiles[b] = (QT, KT, VT, Vaug, yTn)

        def qkv_steps(b):
            """Generator emitting qkv + V-transpose for batch b in small
            steps, so the caller can interleave them into the attention
            instruction stream (filling PE gaps left by the ACT-bound
            exp chain)."""
            QT, KT, VT, Vaug, _ = tiles[b]
            for nt in range(NT):
                xt = xp.tile([128, KO, 512], rdt, tag="xt")
                nc.sync.dma_start(xt[:], xT_r[:, :, ds(b * s + nt * 512, 512)])
                yield
                for Wsb, dst, bias_sb, scale in (
                    (Wq_sb, QT, bq_sb, 0.125),
                    (Wk_sb, KT, bk_sb, 1.0),
                    (Wv_sb, VT, bv_sb, 1.0),
                ):
                    ps = aux_ps.tile([128, 512], f32, tag="aux")
                    for ko in range(KO):
                        nc.tensor.matmul(
                            ps[:],
                            lhsT=Wsb[:, ko],
                            rhs=xt[:, ko],
                            start=(ko == 0),
                            stop=(ko == KO - 1),
                        )
                        if ko % 4 == 3:
                            yield
                    nc.scalar.activation(
                        dst[:, ts(nt, 512)],
                        ps[:],
                        AF.Identity,
                        bias=bias_sb[:],
                        scale=scale,
                    )
                    yield
                for tt in range(nt * 4, nt * 4 + 4):
                    for h in range(HPC):
                        tp = aux_ps.tile([128, D], rdt, tag="aux")
                        nc.tensor.transpose(
                            tp[:],
                            VT[ds(h * D, D), ts(tt, 128)],
                            ident[ds(h * D, D), ds(h * D, D)],
                        )
                        nc.any.tensor_copy(
                            out=Vaug[:, tt, ds(h * (D + 1), D)], in_=tp[:]
                        )
                        yield

        alloc_batch(0)
        for _ in qkv_steps(0):
            pass

        for b in range(nb):
            base = b * s
            QT, KT, VT, Vaug, yTn = tiles[b]
            if b + 1 < nb:
                alloc_batch(b + 1)
                next_gen = qkv_steps(b + 1)
            else:
                next_gen = None

            def pump(n):
                if next_gen is not None:
                    for _ in range(n):
                        if next(next_gen, "done") == "done":
                            break

            # ---- causal attention + projection, per 512-wide q block ----
            # Diagonal chunks (m = kc - 4qt >= 0) only touch q columns
            # j >= 128m, so their matmul/exp/mask run on the narrowed
            # [128m:512) slice. Projection runs per q block right after
            # both heads finish, spreading PE work and output DMA.
            for qt in range(NT):
                nchunks = 4 * qt + 4
                # one ytp bank per head; the two heads' scores matmuls are
                # emitted back-to-back so their lhsT base partitions (0 / 64)
                # land in different PE row groups and run concurrently
                ytps = [
                    yt_ps.tile([D + 1, 512], f32, tag="yt", name=f"yt{b}_{qt}_{h}")
                    for h in range(HPC)
                ]
                for kc in range(nchunks):
                    m = kc - 4 * qt
                    off = max(0, 128 * m)
                    w = 512 - off
                    # both heads' scores in one 2-bank psum tile; halves are
                    # bank-aligned so each matmul stays within one bank, and
                    # the pair shares a single exp + single mask-select op
                    sp = mm_ps.tile([128, 2, 512], f32, tag="s")
                    for h in range(HPC):
                        hp = h * D
                        nc.tensor.matmul(
                            sp[:, h, 0:w],
                            lhsT=KT[ds(hp, D), ts(kc, 128)],
                            rhs=QT[ds(hp, D), ds(qt * 512 + off, w)],
                            start=True,
                            stop=True,
                        )
                    et = expp.tile([128, 2, 512], rdt, tag="exp")
                    nc.scalar.activation(
                        et[:, :, 0:w], sp[:, :, 0:w], AF.Exp
                    )
                    if m >= 0:
                        # keep where (j - i) >= 0 on the narrowed slice of
                        # each half — the causal mask after the 128m shift
                        nc.gpsimd.affine_select(
                            et[:, :, 0:w],
                            et[:, :, 0:w],
                            pattern=[[0, HPC], [1, w]],
                            compare_op=mybir.AluOpType.is_ge,
                            fill=0.0,
                            base=0,
                            channel_multiplier=-1,
                        )
                    for h in range(HPC):
                        nc.tensor.matmul(
                            ytps[h][:, ds(off, w)],
                            lhsT=Vaug[:, kc, ds(h * (D + 1), D + 1)],
                            rhs=et[:, h, 0:w],
                            start=(kc == 0),
                            stop=(kc == nchunks - 1),
                        )
                    pump(2)
                for h in range(HPC):
                    hp = h * D
                    ytp = ytps[h]
                    # normalize: recip of sum row (rounded to f32r),
                    # replicate across 64 partitions with a ones-matmul
                    rec = nrm.tile([D + 1, 512], rdt, tag="rec")
                    with nc.allow_low_precision(
                        reason="recip rounded to f32r feeds the replication matmul"
                    ):
                        nc.vector.reciprocal(rec[ds(D, 1), :], ytp[ds(D, 1), :])
                    rep_ps = aux_ps.tile([D, 512], f32, tag="aux")
                    nc.tensor.matmul(
                        rep_ps[:],
                        lhsT=ones64r[ds(D, 1), :],
                        rhs=rec[ds(D, 1), :],
                        start=True,
                        stop=True,
                    )
                    rep = nrm.tile([D, 512], f32, tag="rep")
                    nc.any.tensor_copy(out=rep[:], in_=rep_ps[:])
                    nc.vector.tensor_mul(
                        yTn[ds(hp, D), ts(qt, 512)], ytp[0:D, :], rep[:]
                    )

                # projection partial for this q block's 4 token tiles
                for tt in range(qt * 4, qt * 4 + 4):
                    ob = obp.tile([128, E], f32, tag="ob")
                    for n in range(E // 512):
                        op = aux_ps.tile([128, 512], f32, tag="aux")
                        nc.tensor.matmul(
                            op[:],
                            lhsT=yTn[:, ts(tt, 128)],
                            rhs=Wp_sb[:, ts(n, 512)],
                            start=True,
                            stop=True,
                        )
                        nc.vector.tensor_copy(ob[:, ts(n, 512)], op[:])
                    nc.sync.dma_start(out[ds(base + tt * 128, 128), :], ob[:])

    return nc


def make_masks():
    i = np.arange(128)[:, None]
    j = np.arange(512)[None, :]
    return np.stack(
        [(j >= i + 128 * m).astype(np.float32) for m in range(4)], axis=0
    )


def shard_inputs(x, W_attn, b_attn, W_proj, nb, s):
    """Build the per-core input maps."""
    E = N_EMBD
    T = nb * s
    x2d = np.ascontiguousarray(x.reshape(T, E), dtype=np.float32)
    xT = np.ascontiguousarray(x2d.T)
    in_maps = []
    for c in range(N_CORES):
        lo = c * HPC * D
        hi = lo + HPC * D
        in_maps.append(
            {
                "xT": xT,
                "Wq": np.ascontiguousarray(W_attn[:, lo:hi]),
                "Wk": np.ascontiguousarray(W_attn[:, E + lo : E + hi]),
                "Wv": np.ascontiguousarray(W_attn[:, 2 * E + lo : 2 * E + hi]),
                "bq": np.ascontiguousarray(b_attn[lo:hi]) * 0.125,
                "bk": np.ascontiguousarray(b_attn[E + lo : E + hi]),
                "bv": np.ascontiguousarray(b_attn[2 * E + lo : 2 * E + hi]),
                "Wp": np.ascontiguousarray(W_proj[lo:hi, :]),
            }
        )
    return in_maps


_NC_CACHE = {}


def _get_nc(nb, s):
    key = (nb, s)
    if key not in _NC_CACHE:
        _NC_CACHE[key] = build_nc(nb, s)
    return _NC_CACHE[key]


def kernel(x, W_attn, b_attn, W_proj, b_proj, _trace=False):
    from concourse.bass_utils import run_bass_kernel_spmd

    nb, s, E = x.shape
    assert E == N_EMBD
    nc = _get_nc(nb, s)
    in_maps = shard_inputs(x, W_attn, b_attn, W_proj, nb, s)
    res = run_bass_kernel_spmd(nc, in_maps, list(range(N_CORES)), trace=_trace)
    acc = res.results[0]["out"].astype(np.float32)
    for c in range(1, N_CORES):
        acc += res.results[c]["out"]
    acc += b_proj.astype(np.float32)
    out = acc.reshape(nb, s, E)
    kernel.last_results = res
    return out
